# revision 36
# baseline (speedup 1.0000x reference)
"""Trainium2 Bass kernel for nn_BiLSTMCell (graph-LSTM cell).

Math (per batch row):
    g_pre[g] = x @ Wx[g].T + hidden @ Wh[g].T + neighbors @ Wn[g].T + b[g]
    i, f, o = sigmoid(g_pre[0..2]);  s = tanh(g_pre[3])
    next_cell = f * cell + i * s
    next_hidden = o * tanh(next_cell)

Strategy: data-parallel over the batch (8192 -> 1024 rows/core on 8 cores),
weights replicated. The x/hidden operands are fused on host into one
A = [x | hidden] with K = 2048 = 16*128, so each gate pre-activation is a
single 16-step accumulating PE matmul chain:
    g_pre[g]^T = W_all[g] @ A^T      ([128k,128h]^T @ [128k,512b] per step)

All heavy operands (W, A, neighbor term, cell, outputs) are float16:
 - fp16 matmul runs at the same 1 cycle/row PE rate as f32r but halves
   HBM traffic and LDWEIGHTS cost (measured ~98ns vs ~189ns on HW).
 - fp16 precision is ample: absmax error in g is ~2e-3 vs the 2e-2 gate.

DMA wire rate scales with per-partition line length (~150GB/s at 1KB
lines, ~250 at 4KB, ~400 at 16KB) and each trigger costs ~640ns of queue
issue time, so every tensor is laid out for long contiguous lines: W0 and
the A bb=0 half stream as 4-ktile chunks (4KB lines) on two queues
concurrently while hh=0's gate chains run kk-interleaved (one chunk pair
feeds 16 matmuls); A bb=1 and W1..W7 are single whole-tile triggers
(16KB lines) that the sync queue reaches by ~8us.

The rank-4 neighbor term (neighbors @ Wn[g].T + b[g], 0.27 GFLOP) is
computed on the host and shipped as an fp16 addend; it joins the
pre-activation via one VectorE add per gate.

Outputs are produced transposed/tiled in fp16 and unscrambled on the host.
"""

import os
import sys

import numpy as np


def _import_concourse():
    try:
        import concourse.bass  # noqa: F401
        return
    except ImportError:
        pass
    for p in ("/opt/trn_rl_repo", "/root/.axon_site/_ro/trn_rl_repo"):
        if os.path.isdir(p) and p not in sys.path:
            sys.path.insert(0, p)
    import concourse.bass  # noqa: F401


B, IN, H, NB, G = 8192, 1024, 1024, 4, 4
NCORES = 8
BS = B // NCORES        # 1024 batch rows per core
KT = 16                 # k-tiles of 128 (IN + H = 2048)
HT = H // 128           # 8 h-tiles of 128
BT = BS // 512          # 2 b-tiles of 512


def _split_excess_waits(nc, max_waits=1, drain_max=0):
    """This walrus build's codegen supports very few sync-wait commands per
    instruction (1 for most ops, 0 spare on Drain). Hoist excess sem-waits
    onto preceding wait-only NoOps on the same engine (AND-semantics over
    monotone semaphores makes sequential waiting equivalent)."""
    from concourse import mybir

    uid = [0]
    n_split = 0
    for fn in nc.m.functions:
        for bb in fn.blocks:
            new_insts = []
            for inst in bb.instructions:
                limit = drain_max if type(inst).__name__ == "InstDrain" else max_waits
                si = inst.sync_info
                waits = list(si.on_wait) if si and si.on_wait else []
                if len(waits) > limit:
                    n_split += 1
                    if limit > 0:
                        excess, keep = waits[:-limit], waits[-limit:]
                    else:
                        excess, keep = waits, []
                    for i in range(0, len(excess), max_waits):
                        chunk = excess[i:i + max_waits]
                        nop = mybir.InstNoOp(
                            name=f"waitsplit_{uid[0]}",
                            sync_info=mybir.SyncInfo(on_wait=chunk, on_update=[]),
                        )
                        uid[0] += 1
                        nop.engine = inst.engine
                        new_insts.append(nop)
                    si.on_wait = keep
                    inst.sync_info = si
                new_insts.append(inst)
            bb.instructions = new_insts
    return n_split


_PROG = None

# gate order (s, i, f, o): the deep tanh(s)/mul chain starts while the
# later gates' matmuls still stream, and the final o-gate leaves only a
# short sigmoid+mul tail after the very last matmul.
GORDER = (3, 0, 1, 2)


def _build_program():
    import concourse.bass as bass
    import concourse.tile as tile
    from concourse import mybir

    f32 = mybir.dt.float32
    f16 = mybir.dt.float16
    ACT = mybir.ActivationFunctionType

    nc = bass.Bass()
    at_d = nc.dram_tensor("AT", [BT, 128, KT, 512], f16, kind="ExternalInput")
    w_d = nc.dram_tensor("W", [HT, 128, KT, G * 128], f16, kind="ExternalInput")
    ct_d = nc.dram_tensor("CT", [HT, 128, BT * 512], f16, kind="ExternalInput")
    nb_d = nc.dram_tensor("NBT", [HT, BT, 128, G, 512], f16, kind="ExternalInput")
    bias_d = nc.dram_tensor("BIAS", [128, 1], f32, kind="ExternalInput")
    ho_d = nc.dram_tensor("hT", [HT, BT, 128, 512], f16, kind="ExternalOutput")
    co_d = nc.dram_tensor("cT", [HT, BT, 128, 512], f16, kind="ExternalOutput")

    with tile.TileContext(nc) as tc:
        with (
            tc.tile_pool(name="at", bufs=1) as p_at,
            tc.tile_pool(name="w", bufs=5) as p_w,
            tc.tile_pool(name="cell", bufs=3) as p_cell,
            tc.tile_pool(name="nb", bufs=3) as p_nb,
            tc.tile_pool(name="bias", bufs=1) as p_bias,
            tc.tile_pool(name="eps", bufs=2) as p_eps,
            tc.tile_pool(name="outs", bufs=2) as p_out,
            tc.tile_pool(name="ps", bufs=8, space="PSUM") as p_ps,
        ):
            # zero column for the ACT bias port (real bias is folded into the
            # host-computed neighbor term). A float bias would make the
            # framework stage a const tensor via a TENSOR_LOAD that delays
            # the sync queue's first DMA at the critical head.
            bias_t = p_bias.tile([128, 1], f32, name="bias_t")
            nc.gpsimd.dma_start(bias_t[:], bias_d[:])
            zcol = bias_t[:, 0:1]

            # PE p-state warm-up: the PE runs the first ~3.3us of real work
            # at 1.2GHz (measured 427ns -> 216ns matmul spacing). Full-size
            # N=512 matmuls on a never-written scratch tile (garbage values,
            # zero input deps) draw real array power from the preamble end
            # through the DMA-priming window, finishing the DVFS ramp before
            # data lands. Output goes to a scratch psum tile that is never
            # read; any NaNs are discarded by the real chains' start=True.
            warm_in = p_bias.tile([128, 640], f16, name="warm_in")
            # memset from the scalar queue: its body starts ~4us earlier
            # than gpsimd's, so the warm stream spans ~6-12.5us and ends
            # right as the first real chunks land instead of overshooting
            nc.vector.memset(warm_in[:], 1.0)
            warm_ps = p_ps.tile([128, 512], f32, name="warm", tag="ps")
            for _ in range(22):
                nc.tensor.matmul(
                    warm_ps[:], warm_in[:, 0:128], warm_in[:, 128:640],
                    start=True, stop=True,
                )

            at = p_at.tile([128, BT, KT, 512], f16, name="at")
            wts = []
            for hh in range(HT):
                wts.append(p_w.tile([128, KT, G * 128], f16, name="wt", tag="wt"))

            # DMA wire rate scales with per-partition line length (~150GB/s
            # at 1KB lines, ~250 at 4KB, ~400 at 16KB), and each trigger
            # costs ~640ns of queue issue time. So the head moves in 4-ktile
            # chunks (4KB lines): W0 from sync and AT bb=0 from scalar
            # CONCURRENTLY (transfers share one wire, but trigger issue and
            # ring priming parallelize), then AT bb=1 as ONE 16KB-line
            # trigger, then the whole-tile W1..W7 (16KB lines) which the
            # sync queue now reaches by ~8us instead of ~25us (late W tiles
            # were recurring multi-us tensor stalls). hh=0's groups run
            # their four gate chains kk-INTERLEAVED so one 4-ktile chunk
            # pair feeds 16 matmuls and the head never starves.
            # Cell/neighbor ride the scalar queue: per-hh cell chunks (2KB
            # lines) and per-group neighbor chunks (4KB lines).
            for c in range(4):
                nc.sync.dma_start(
                    wts[0][:, 4 * c:4 * c + 4, :], w_d[0, :, 4 * c:4 * c + 4, :]
                )
                nc.scalar.dma_start(
                    at[:, 0, 4 * c:4 * c + 4, :], at_d[0, :, 4 * c:4 * c + 4, :]
                )
            nc.sync.dma_start(at[:, 1], at_d[1])
            for hh in range(1, HT):
                nc.sync.dma_start(wts[hh][:], w_d[hh])

            for hh in range(HT):
                wt = wts[hh]

                ctt = p_cell.tile([128, BT * 512], f16, name="ct", tag="ct")
                nc.scalar.dma_start(ctt[:], ct_d[hh])

                for bb in range(BT):
                    ct = ctt[:, bb * 512:(bb + 1) * 512]
                    nbt = p_nb.tile([128, G, 512], f16, name="nbt", tag="nbt")
                    nc.scalar.dma_start(nbt[:], nb_d[hh, bb])

                    ps = [None] * G
                    for g in GORDER:
                        ps[g] = p_ps.tile([128, 512], f32, name=f"pt{g}", tag="ps")
                    if hh == 0:
                        # kk-interleaved: paced to the arriving DMA chunks
                        for kk in range(KT):
                            for g in GORDER:
                                nc.tensor.matmul(
                                    ps[g][:],
                                    wt[:, kk, g * 128:(g + 1) * 128],
                                    at[:, bb, kk, :],
                                    start=(kk == 0),
                                    stop=(kk == KT - 1),
                                )
                    else:
                        # gate-sequential: the s-chain finishes early so its
                        # deep tanh/mul chain overlaps the later chains
                        for g in GORDER:
                            for kk in range(KT):
                                nc.tensor.matmul(
                                    ps[g][:],
                                    wt[:, kk, g * 128:(g + 1) * 128],
                                    at[:, bb, kk, :],
                                    start=(kk == 0),
                                    stop=(kk == KT - 1),
                                )

                    last_group = hh == HT - 1 and bb == BT - 1

                    def pre(g, name, sl, w):
                        # pre-activation = psum + (neighbor term + bias)
                        t = p_eps.tile([128, w], f16, name=name, tag=name)
                        nc.vector.tensor_add(t[:], ps[g][:, sl], nbt[:, g, sl])
                        return t

                    def sif_stage(sl):
                        # everything that depends only on the s/i/f chains;
                        # runs while the o-gate matmuls still stream.
                        w = sl.stop - sl.start
                        tan_s = pre(3, "tan_s", sl, w)
                        nc.scalar.activation(tan_s[:], tan_s[:], ACT.Tanh, bias=zcol)
                        sig_i = pre(0, "sig_i", sl, w)
                        nc.scalar.activation(sig_i[:], sig_i[:], ACT.Sigmoid, bias=zcol)
                        sig_f = pre(1, "sig_f", sl, w)
                        nc.scalar.activation(sig_f[:], sig_f[:], ACT.Sigmoid, bias=zcol)

                        t_is = p_eps.tile([128, w], f16, name="t_is", tag="t_is")
                        nc.vector.tensor_mul(t_is[:], sig_i[:], tan_s[:])
                        t_fc = p_eps.tile([128, w], f16, name="t_fc", tag="t_fc")
                        nc.vector.tensor_mul(t_fc[:], sig_f[:], ct[:, sl])
                        c_new = p_out.tile([128, w], f16, name="c_new", tag="c_new")
                        nc.vector.tensor_add(c_new[:], t_is[:], t_fc[:])
                        # last-group outputs flush on the sync queue, which
                        # is idle by then; a scalar-queue trigger (~600ns)
                        # would block the tail ACT chain.
                        q = nc.sync if last_group else nc.gpsimd
                        q.dma_start(co_d[hh, bb][:, sl], c_new[:])
                        tan_c = p_eps.tile([128, w], f16, name="tan_c", tag="tan_c")
                        nc.scalar.activation(tan_c[:], c_new[:], ACT.Tanh, bias=zcol)
                        return tan_c

                    def o_stage(sl, tan_c, off, sig_o=None, outq=None):
                        w = sl.stop - sl.start
                        if sig_o is None:
                            sig_o = pre(2, "sig_o", sl, w)
                            nc.scalar.activation(
                                sig_o[:], sig_o[:], ACT.Sigmoid, bias=zcol
                            )
                        h_new = p_out.tile([128, w], f16, name="h_new", tag="h_new")
                        nc.vector.tensor_mul(
                            h_new[:], sig_o[:, 0:w], tan_c[:, off:off + w]
                        )
                        if outq is None:
                            outq = nc.sync if last_group else nc.gpsimd
                        outq.dma_start(ho_d[hh, bb][:, sl], h_new[:])

                    if last_group:
                        # Tail: emit the full-width s/i/f stage first (it
                        # overlaps the o-gate matmuls), then the o stage in
                        # halves so the post-last-matmul critical path
                        # (pre_o -> sigmoid -> mul -> store) pipelines.
                        tan_c = sif_stage(slice(0, 512))
                        o_stage(slice(0, 256), tan_c, 0)
                        o_stage(slice(256, 512), tan_c, 256)
                    else:
                        # psum-freeing pre-adds ALL come first on the DVE
                        # queue: bank recycling for group i+2 then never
                        # waits on this group's ACT/mul chain.
                        sl = slice(0, 512)
                        tan_s = pre(3, "tan_s", sl, 512)
                        sig_i = pre(0, "sig_i", sl, 512)
                        sig_f = pre(1, "sig_f", sl, 512)
                        sig_o = pre(2, "sig_o", sl, 512)
                        nc.scalar.activation(tan_s[:], tan_s[:], ACT.Tanh, bias=zcol)
                        nc.scalar.activation(sig_i[:], sig_i[:], ACT.Sigmoid, bias=zcol)
                        nc.scalar.activation(sig_f[:], sig_f[:], ACT.Sigmoid, bias=zcol)
                        nc.scalar.activation(sig_o[:], sig_o[:], ACT.Sigmoid, bias=zcol)

                        t_is = p_eps.tile([128, 512], f16, name="t_is", tag="t_is")
                        nc.vector.tensor_mul(t_is[:], sig_i[:], tan_s[:])
                        t_fc = p_eps.tile([128, 512], f16, name="t_fc", tag="t_fc")
                        nc.vector.tensor_mul(t_fc[:], sig_f[:], ct[:, sl])
                        c_new = p_out.tile([128, 512], f16, name="c_new", tag="c_new")
                        nc.vector.tensor_add(c_new[:], t_is[:], t_fc[:])
                        nc.gpsimd.dma_start(co_d[hh, bb][:, sl], c_new[:])
                        tan_c = p_eps.tile([128, 512], f16, name="tan_c", tag="tan_c")
                        nc.scalar.activation(tan_c[:], c_new[:], ACT.Tanh, bias=zcol)
                        o_stage(sl, tan_c, 0, sig_o=sig_o)

    _split_excess_waits(nc)
    return nc


def _get_program():
    global _PROG
    if _PROG is None:
        _PROG = _build_program()
    return _PROG


def _prep_inputs(x, hidden, cell, neighbors, Wx, Wh, Wn, b):
    """Host-side shard/relayout. Returns per-core input maps."""
    x = np.asarray(x, np.float32)
    hidden = np.asarray(hidden, np.float32)
    cell = np.asarray(cell, np.float32)
    neighbors = np.asarray(neighbors, np.float32)
    Wx = np.asarray(Wx, np.float32)
    Wh = np.asarray(Wh, np.float32)
    Wn = np.asarray(Wn, np.float32)
    b = np.asarray(b, np.float32)

    # A = [x | hidden]: K = 2048 exactly.
    A = np.concatenate([x, hidden], axis=1)
    W_all = np.concatenate([Wx, Wh], axis=2)  # [G, H, 2048]

    # SBUF weight layout, kk-major: [hh, p(k), kk, g*128 + j(h)]  (fp16)
    w_host = np.ascontiguousarray(
        W_all.reshape(G, HT, 128, KT, 128).transpose(1, 4, 3, 0, 2)
    ).reshape(HT, 128, KT, G * 128).astype(np.float16)

    # neighbor term + bias, [G, B, H] computed on host in f64 -> fp16
    nbterm = (
        np.einsum(
            "bj,ghj->gbh", neighbors.astype(np.float64), Wn.astype(np.float64)
        )
        + b.astype(np.float64)[:, None, :]
    ).astype(np.float32)

    bias_host = np.zeros((128, 1), np.float32)

    in_maps = []
    for c in range(NCORES):
        sl = slice(c * BS, (c + 1) * BS)
        # A^T tiled, bb-major: [bb, p(k), kk, n(b)]  (fp16)
        at_host = np.ascontiguousarray(
            A[sl].reshape(BT, 512, KT, 128).transpose(0, 3, 2, 1)
        ).astype(np.float16)
        # cell^T tiled: [hh, j(h), b]  (fp16)
        ct_host = np.ascontiguousarray(
            cell[sl].T.reshape(HT, 128, BS)
        ).astype(np.float16)
        # neighbor term tiled: [hh, bb, j(h), g, n(b)]  (fp16)
        nb_host = np.ascontiguousarray(
            nbterm[:, sl, :].transpose(2, 1, 0)  # [H, BS, G]
            .reshape(HT, 128, BT, 512, G)
            .transpose(0, 2, 1, 4, 3)            # [hh, bb, j, g, n]
        ).astype(np.float16)
        in_maps.append(
            {
                "AT": at_host,
                "W": w_host,
                "CT": ct_host,
                "NBT": nb_host,
                "BIAS": bias_host,
            }
        )
    return in_maps


def _gather_outputs(results):
    """Invert the per-core [HT, BT, 128, 512] transposed fp16 tiling."""
    h_parts, c_parts = [], []
    for c in range(NCORES):
        hT = np.asarray(results[c]["hT"]).astype(np.float32)
        cT = np.asarray(results[c]["cT"]).astype(np.float32)
        # [hh, bb, j, n] -> [hh*128+j, bb*512+n] -> transpose to [b, h]
        h_parts.append(hT.transpose(0, 2, 1, 3).reshape(H, BS).T)
        c_parts.append(cT.transpose(0, 2, 1, 3).reshape(H, BS).T)
    next_hidden = np.ascontiguousarray(np.concatenate(h_parts, axis=0), dtype=np.float32)
    next_cell = np.ascontiguousarray(np.concatenate(c_parts, axis=0), dtype=np.float32)
    return next_hidden, next_cell


def _run(in_maps, trace=False, tmpdir=None):
    _import_concourse()
    from concourse.bass_utils import run_bass_kernel_spmd

    if trace:
        _install_ntff_shim()
    nc = _get_program()
    last_err = None
    for attempt in range(3):
        try:
            return run_bass_kernel_spmd(
                nc, in_maps, list(range(NCORES)), trace=trace, tmpdir=tmpdir
            )
        except Exception as e:  # transient device wedge: retry
            last_err = e
            if "UNRECOVERABLE" not in str(e) and "NRT" not in str(e):
                raise
    raise last_err


def _install_ntff_shim():
    """Shim antenv.axon_hooks (absent in this image) so trace=True works."""
    import types

    if "antenv.axon_hooks" not in sys.modules:
        mod = types.ModuleType("antenv.axon_hooks")
        mod._hook = None
        mod.set_axon_ntff_profile_hook = lambda h: setattr(mod, "_hook", h)
        mod.get_axon_ntff_profile_hook = lambda: mod._hook
        sys.modules["antenv.axon_hooks"] = mod
        try:
            import antenv
            antenv.axon_hooks = mod
        except ImportError:
            pass
    mod = sys.modules["antenv.axon_hooks"]
    if mod._hook is None:
        from trn_agent_boot.trn_boot import _ntff_profile_via_ctypes
        mod._hook = _ntff_profile_via_ctypes("/opt/axon/libaxon_pjrt.so")
    from concourse import bass_utils
    bass_utils.upload_artifacts = lambda tmpdir: f"local:{tmpdir}"


def kernel(x, hidden, cell, neighbors, Wx, Wh, Wn, b):
    _import_concourse()
    in_maps = _prep_inputs(x, hidden, cell, neighbors, Wx, Wh, Wn, b)
    res = _run(in_maps, trace=False)
    return _gather_outputs(res.results)


# revision 37
# speedup vs baseline: 1.0001x; 1.0001x over previous
"""Trainium2 Bass kernel for nn_BiLSTMCell (graph-LSTM cell).

Math (per batch row):
    g_pre[g] = x @ Wx[g].T + hidden @ Wh[g].T + neighbors @ Wn[g].T + b[g]
    i, f, o = sigmoid(g_pre[0..2]);  s = tanh(g_pre[3])
    next_cell = f * cell + i * s
    next_hidden = o * tanh(next_cell)

Strategy: data-parallel over the batch (8192 -> 1024 rows/core on 8 cores),
weights replicated. The x/hidden operands are fused on host into one
A = [x | hidden] with K = 2048 = 16*128, so each gate pre-activation is a
single 16-step accumulating PE matmul chain:
    g_pre[g]^T = W_all[g] @ A^T      ([128k,128h]^T @ [128k,512b] per step)

All heavy operands (W, A, neighbor term, cell, outputs) are float16:
 - fp16 matmul runs at the same 1 cycle/row PE rate as f32r but halves
   HBM traffic and LDWEIGHTS cost (measured ~98ns vs ~189ns on HW).
 - fp16 precision is ample: absmax error in g is ~2e-3 vs the 2e-2 gate.

DMA wire rate scales with per-partition line length (~150GB/s at 1KB
lines, ~250 at 4KB, ~400 at 16KB) and each trigger costs ~640ns of queue
issue time, so every tensor is laid out for long contiguous lines: W0 and
the A bb=0 half stream as 4-ktile chunks (4KB lines) on two queues
concurrently while hh=0's gate chains run kk-interleaved (one chunk pair
feeds 16 matmuls); A bb=1 and W1..W7 are single whole-tile triggers
(16KB lines) that the sync queue reaches by ~8us.

The rank-4 neighbor term (neighbors @ Wn[g].T + b[g], 0.27 GFLOP) is
computed on the host and shipped as an fp16 addend; it joins the
pre-activation via one VectorE add per gate.

Outputs are produced transposed/tiled in fp16 and unscrambled on the host.
"""

import os
import sys

import numpy as np


def _import_concourse():
    try:
        import concourse.bass  # noqa: F401
        return
    except ImportError:
        pass
    for p in ("/opt/trn_rl_repo", "/root/.axon_site/_ro/trn_rl_repo"):
        if os.path.isdir(p) and p not in sys.path:
            sys.path.insert(0, p)
    import concourse.bass  # noqa: F401


B, IN, H, NB, G = 8192, 1024, 1024, 4, 4
NCORES = 8
BS = B // NCORES        # 1024 batch rows per core
KT = 16                 # k-tiles of 128 (IN + H = 2048)
HT = H // 128           # 8 h-tiles of 128
BT = BS // 512          # 2 b-tiles of 512


def _split_excess_waits(nc, max_waits=1, drain_max=0):
    """This walrus build's codegen supports very few sync-wait commands per
    instruction (1 for most ops, 0 spare on Drain). Hoist excess sem-waits
    onto preceding wait-only NoOps on the same engine (AND-semantics over
    monotone semaphores makes sequential waiting equivalent)."""
    from concourse import mybir

    uid = [0]
    n_split = 0
    for fn in nc.m.functions:
        for bb in fn.blocks:
            new_insts = []
            for inst in bb.instructions:
                limit = drain_max if type(inst).__name__ == "InstDrain" else max_waits
                si = inst.sync_info
                waits = list(si.on_wait) if si and si.on_wait else []
                if len(waits) > limit:
                    n_split += 1
                    if limit > 0:
                        excess, keep = waits[:-limit], waits[-limit:]
                    else:
                        excess, keep = waits, []
                    for i in range(0, len(excess), max_waits):
                        chunk = excess[i:i + max_waits]
                        nop = mybir.InstNoOp(
                            name=f"waitsplit_{uid[0]}",
                            sync_info=mybir.SyncInfo(on_wait=chunk, on_update=[]),
                        )
                        uid[0] += 1
                        nop.engine = inst.engine
                        new_insts.append(nop)
                    si.on_wait = keep
                    inst.sync_info = si
                new_insts.append(inst)
            bb.instructions = new_insts
    return n_split


_PROG = None

# gate order (s, i, f, o): the deep tanh(s)/mul chain starts while the
# later gates' matmuls still stream, and the final o-gate leaves only a
# short sigmoid+mul tail after the very last matmul.
GORDER = (3, 0, 1, 2)


def _build_program():
    import concourse.bass as bass
    import concourse.tile as tile
    from concourse import mybir

    f32 = mybir.dt.float32
    f16 = mybir.dt.float16
    ACT = mybir.ActivationFunctionType

    nc = bass.Bass()
    at_d = nc.dram_tensor("AT", [BT, 128, KT, 512], f16, kind="ExternalInput")
    w_d = nc.dram_tensor("W", [HT, 128, KT, G * 128], f16, kind="ExternalInput")
    ct_d = nc.dram_tensor("CT", [HT, 128, BT * 512], f16, kind="ExternalInput")
    nb_d = nc.dram_tensor("NBT", [HT, BT, 128, G, 512], f16, kind="ExternalInput")
    bias_d = nc.dram_tensor("BIAS", [128, 1], f32, kind="ExternalInput")
    ho_d = nc.dram_tensor("hT", [HT, BT, 128, 512], f16, kind="ExternalOutput")
    co_d = nc.dram_tensor("cT", [HT, BT, 128, 512], f16, kind="ExternalOutput")

    with tile.TileContext(nc) as tc:
        with (
            tc.tile_pool(name="at", bufs=1) as p_at,
            tc.tile_pool(name="w", bufs=5) as p_w,
            tc.tile_pool(name="cell", bufs=3) as p_cell,
            tc.tile_pool(name="nb", bufs=3) as p_nb,
            tc.tile_pool(name="bias", bufs=1) as p_bias,
            tc.tile_pool(name="eps", bufs=2) as p_eps,
            tc.tile_pool(name="outs", bufs=2) as p_out,
            tc.tile_pool(name="ps", bufs=8, space="PSUM") as p_ps,
        ):
            # zero column for the ACT bias port (real bias is folded into the
            # host-computed neighbor term). A float bias would make the
            # framework stage a const tensor via a TENSOR_LOAD that delays
            # the sync queue's first DMA at the critical head.
            bias_t = p_bias.tile([128, 1], f32, name="bias_t")
            nc.gpsimd.dma_start(bias_t[:], bias_d[:])
            zcol = bias_t[:, 0:1]

            at = p_at.tile([128, BT, KT, 512], f16, name="at")
            wts = []
            for hh in range(HT):
                wts.append(p_w.tile([128, KT, G * 128], f16, name="wt", tag="wt"))

            # DMA wire rate scales with per-partition line length (~150GB/s
            # at 1KB lines, ~250 at 4KB, ~400 at 16KB), and each trigger
            # costs ~640ns of queue issue time. So the head moves in 4-ktile
            # chunks (4KB lines): W0 from sync and AT bb=0 from scalar
            # CONCURRENTLY (transfers share one wire, but trigger issue and
            # ring priming parallelize), then AT bb=1 as ONE 16KB-line
            # trigger, then the whole-tile W1..W7 (16KB lines) which the
            # sync queue now reaches by ~8us instead of ~25us (late W tiles
            # were recurring multi-us tensor stalls). hh=0's groups run
            # their four gate chains kk-INTERLEAVED so one 4-ktile chunk
            # pair feeds 16 matmuls and the head never starves.
            # Cell/neighbor ride the scalar queue: per-hh cell chunks (2KB
            # lines) and per-group neighbor chunks (4KB lines).
            for c in range(4):
                nc.sync.dma_start(
                    wts[0][:, 4 * c:4 * c + 4, :], w_d[0, :, 4 * c:4 * c + 4, :]
                )
                nc.scalar.dma_start(
                    at[:, 0, 4 * c:4 * c + 4, :], at_d[0, :, 4 * c:4 * c + 4, :]
                )
            nc.sync.dma_start(at[:, 1], at_d[1])
            for hh in range(1, HT):
                nc.sync.dma_start(wts[hh][:], w_d[hh])

            for hh in range(HT):
                wt = wts[hh]

                ctt = p_cell.tile([128, BT * 512], f16, name="ct", tag="ct")
                nc.scalar.dma_start(ctt[:], ct_d[hh])

                for bb in range(BT):
                    ct = ctt[:, bb * 512:(bb + 1) * 512]
                    nbt = p_nb.tile([128, G, 512], f16, name="nbt", tag="nbt")
                    nc.scalar.dma_start(nbt[:], nb_d[hh, bb])

                    ps = [None] * G
                    for g in GORDER:
                        ps[g] = p_ps.tile([128, 512], f32, name=f"pt{g}", tag="ps")
                    if hh == 0:
                        # kk-interleaved: paced to the arriving DMA chunks
                        for kk in range(KT):
                            for g in GORDER:
                                nc.tensor.matmul(
                                    ps[g][:],
                                    wt[:, kk, g * 128:(g + 1) * 128],
                                    at[:, bb, kk, :],
                                    start=(kk == 0),
                                    stop=(kk == KT - 1),
                                )
                    else:
                        # gate-sequential: the s-chain finishes early so its
                        # deep tanh/mul chain overlaps the later chains
                        for g in GORDER:
                            for kk in range(KT):
                                nc.tensor.matmul(
                                    ps[g][:],
                                    wt[:, kk, g * 128:(g + 1) * 128],
                                    at[:, bb, kk, :],
                                    start=(kk == 0),
                                    stop=(kk == KT - 1),
                                )

                    last_group = hh == HT - 1 and bb == BT - 1

                    def pre(g, name, sl, w):
                        # pre-activation = psum + (neighbor term + bias)
                        t = p_eps.tile([128, w], f16, name=name, tag=name)
                        nc.vector.tensor_add(t[:], ps[g][:, sl], nbt[:, g, sl])
                        return t

                    def sif_stage(sl):
                        # everything that depends only on the s/i/f chains;
                        # runs while the o-gate matmuls still stream.
                        w = sl.stop - sl.start
                        tan_s = pre(3, "tan_s", sl, w)
                        nc.scalar.activation(tan_s[:], tan_s[:], ACT.Tanh, bias=zcol)
                        sig_i = pre(0, "sig_i", sl, w)
                        nc.scalar.activation(sig_i[:], sig_i[:], ACT.Sigmoid, bias=zcol)
                        sig_f = pre(1, "sig_f", sl, w)
                        nc.scalar.activation(sig_f[:], sig_f[:], ACT.Sigmoid, bias=zcol)

                        t_is = p_eps.tile([128, w], f16, name="t_is", tag="t_is")
                        nc.vector.tensor_mul(t_is[:], sig_i[:], tan_s[:])
                        t_fc = p_eps.tile([128, w], f16, name="t_fc", tag="t_fc")
                        nc.vector.tensor_mul(t_fc[:], sig_f[:], ct[:, sl])
                        c_new = p_out.tile([128, w], f16, name="c_new", tag="c_new")
                        nc.vector.tensor_add(c_new[:], t_is[:], t_fc[:])
                        # last-group outputs flush on the sync queue, which
                        # is idle by then; a scalar-queue trigger (~600ns)
                        # would block the tail ACT chain.
                        q = nc.sync if last_group else nc.gpsimd
                        q.dma_start(co_d[hh, bb][:, sl], c_new[:])
                        tan_c = p_eps.tile([128, w], f16, name="tan_c", tag="tan_c")
                        nc.scalar.activation(tan_c[:], c_new[:], ACT.Tanh, bias=zcol)
                        return tan_c

                    def o_stage(sl, tan_c, off, sig_o=None, outq=None):
                        w = sl.stop - sl.start
                        if sig_o is None:
                            sig_o = pre(2, "sig_o", sl, w)
                            nc.scalar.activation(
                                sig_o[:], sig_o[:], ACT.Sigmoid, bias=zcol
                            )
                        h_new = p_out.tile([128, w], f16, name="h_new", tag="h_new")
                        nc.vector.tensor_mul(
                            h_new[:], sig_o[:, 0:w], tan_c[:, off:off + w]
                        )
                        if outq is None:
                            outq = nc.sync if last_group else nc.gpsimd
                        outq.dma_start(ho_d[hh, bb][:, sl], h_new[:])

                    if last_group:
                        # Tail: emit the full-width s/i/f stage first (it
                        # overlaps the o-gate matmuls), then the o stage in
                        # halves so the post-last-matmul critical path
                        # (pre_o -> sigmoid -> mul -> store) pipelines.
                        tan_c = sif_stage(slice(0, 512))
                        o_stage(slice(0, 256), tan_c, 0)
                        o_stage(slice(256, 512), tan_c, 256)
                    else:
                        # psum-freeing pre-adds ALL come first on the DVE
                        # queue: bank recycling for group i+2 then never
                        # waits on this group's ACT/mul chain.
                        sl = slice(0, 512)
                        tan_s = pre(3, "tan_s", sl, 512)
                        sig_i = pre(0, "sig_i", sl, 512)
                        sig_f = pre(1, "sig_f", sl, 512)
                        sig_o = pre(2, "sig_o", sl, 512)
                        nc.scalar.activation(tan_s[:], tan_s[:], ACT.Tanh, bias=zcol)
                        nc.scalar.activation(sig_i[:], sig_i[:], ACT.Sigmoid, bias=zcol)
                        nc.scalar.activation(sig_f[:], sig_f[:], ACT.Sigmoid, bias=zcol)
                        nc.scalar.activation(sig_o[:], sig_o[:], ACT.Sigmoid, bias=zcol)

                        t_is = p_eps.tile([128, 512], f16, name="t_is", tag="t_is")
                        nc.vector.tensor_mul(t_is[:], sig_i[:], tan_s[:])
                        t_fc = p_eps.tile([128, 512], f16, name="t_fc", tag="t_fc")
                        nc.vector.tensor_mul(t_fc[:], sig_f[:], ct[:, sl])
                        c_new = p_out.tile([128, 512], f16, name="c_new", tag="c_new")
                        nc.vector.tensor_add(c_new[:], t_is[:], t_fc[:])
                        nc.gpsimd.dma_start(co_d[hh, bb][:, sl], c_new[:])
                        tan_c = p_eps.tile([128, 512], f16, name="tan_c", tag="tan_c")
                        nc.scalar.activation(tan_c[:], c_new[:], ACT.Tanh, bias=zcol)
                        o_stage(sl, tan_c, 0, sig_o=sig_o)

    _split_excess_waits(nc)
    return nc


def _get_program():
    global _PROG
    if _PROG is None:
        _PROG = _build_program()
    return _PROG


def _prep_inputs(x, hidden, cell, neighbors, Wx, Wh, Wn, b):
    """Host-side shard/relayout. Returns per-core input maps."""
    x = np.asarray(x, np.float32)
    hidden = np.asarray(hidden, np.float32)
    cell = np.asarray(cell, np.float32)
    neighbors = np.asarray(neighbors, np.float32)
    Wx = np.asarray(Wx, np.float32)
    Wh = np.asarray(Wh, np.float32)
    Wn = np.asarray(Wn, np.float32)
    b = np.asarray(b, np.float32)

    # A = [x | hidden]: K = 2048 exactly.
    A = np.concatenate([x, hidden], axis=1)
    W_all = np.concatenate([Wx, Wh], axis=2)  # [G, H, 2048]

    # SBUF weight layout, kk-major: [hh, p(k), kk, g*128 + j(h)]  (fp16)
    w_host = np.ascontiguousarray(
        W_all.reshape(G, HT, 128, KT, 128).transpose(1, 4, 3, 0, 2)
    ).reshape(HT, 128, KT, G * 128).astype(np.float16)

    # neighbor term + bias, [G, B, H] computed on host in f64 -> fp16
    nbterm = (
        np.einsum(
            "bj,ghj->gbh", neighbors.astype(np.float64), Wn.astype(np.float64)
        )
        + b.astype(np.float64)[:, None, :]
    ).astype(np.float32)

    bias_host = np.zeros((128, 1), np.float32)

    in_maps = []
    for c in range(NCORES):
        sl = slice(c * BS, (c + 1) * BS)
        # A^T tiled, bb-major: [bb, p(k), kk, n(b)]  (fp16)
        at_host = np.ascontiguousarray(
            A[sl].reshape(BT, 512, KT, 128).transpose(0, 3, 2, 1)
        ).astype(np.float16)
        # cell^T tiled: [hh, j(h), b]  (fp16)
        ct_host = np.ascontiguousarray(
            cell[sl].T.reshape(HT, 128, BS)
        ).astype(np.float16)
        # neighbor term tiled: [hh, bb, j(h), g, n(b)]  (fp16)
        nb_host = np.ascontiguousarray(
            nbterm[:, sl, :].transpose(2, 1, 0)  # [H, BS, G]
            .reshape(HT, 128, BT, 512, G)
            .transpose(0, 2, 1, 4, 3)            # [hh, bb, j, g, n]
        ).astype(np.float16)
        in_maps.append(
            {
                "AT": at_host,
                "W": w_host,
                "CT": ct_host,
                "NBT": nb_host,
                "BIAS": bias_host,
            }
        )
    return in_maps


def _gather_outputs(results):
    """Invert the per-core [HT, BT, 128, 512] transposed fp16 tiling."""
    h_parts, c_parts = [], []
    for c in range(NCORES):
        hT = np.asarray(results[c]["hT"]).astype(np.float32)
        cT = np.asarray(results[c]["cT"]).astype(np.float32)
        # [hh, bb, j, n] -> [hh*128+j, bb*512+n] -> transpose to [b, h]
        h_parts.append(hT.transpose(0, 2, 1, 3).reshape(H, BS).T)
        c_parts.append(cT.transpose(0, 2, 1, 3).reshape(H, BS).T)
    next_hidden = np.ascontiguousarray(np.concatenate(h_parts, axis=0), dtype=np.float32)
    next_cell = np.ascontiguousarray(np.concatenate(c_parts, axis=0), dtype=np.float32)
    return next_hidden, next_cell


def _run(in_maps, trace=False, tmpdir=None):
    _import_concourse()
    from concourse.bass_utils import run_bass_kernel_spmd

    if trace:
        _install_ntff_shim()
    nc = _get_program()
    last_err = None
    for attempt in range(3):
        try:
            return run_bass_kernel_spmd(
                nc, in_maps, list(range(NCORES)), trace=trace, tmpdir=tmpdir
            )
        except Exception as e:  # transient device wedge: retry
            last_err = e
            if "UNRECOVERABLE" not in str(e) and "NRT" not in str(e):
                raise
    raise last_err


def _install_ntff_shim():
    """Shim antenv.axon_hooks (absent in this image) so trace=True works."""
    import types

    if "antenv.axon_hooks" not in sys.modules:
        mod = types.ModuleType("antenv.axon_hooks")
        mod._hook = None
        mod.set_axon_ntff_profile_hook = lambda h: setattr(mod, "_hook", h)
        mod.get_axon_ntff_profile_hook = lambda: mod._hook
        sys.modules["antenv.axon_hooks"] = mod
        try:
            import antenv
            antenv.axon_hooks = mod
        except ImportError:
            pass
    mod = sys.modules["antenv.axon_hooks"]
    if mod._hook is None:
        from trn_agent_boot.trn_boot import _ntff_profile_via_ctypes
        mod._hook = _ntff_profile_via_ctypes("/opt/axon/libaxon_pjrt.so")
    from concourse import bass_utils
    bass_utils.upload_artifacts = lambda tmpdir: f"local:{tmpdir}"


def kernel(x, hidden, cell, neighbors, Wx, Wh, Wn, b):
    _import_concourse()
    in_maps = _prep_inputs(x, hidden, cell, neighbors, Wx, Wh, Wn, b)
    res = _run(in_maps, trace=False)
    return _gather_outputs(res.results)
